# revision 3
# baseline (speedup 1.0000x reference)
"""Single-head causal attention (B=8, S=2048, D=1024) on 8 Trainium2 cores.

Strategy: pure data-parallel over batch — core b computes attention for
batch element b end-to-end (no collectives). All matmuls run in FP32r
(fp32 with 11-bit mantissa, full PE rate at moving-dim >= 256).

Per-core pipeline:
  Phase A: PE-transpose activations, project K^T (+bk), V, Q^T (+bq, /sqrt(D))
           with f32r matmuls. K^T [e, sk] and V [sk, dv] stay resident in
           SBUF; Q^T bounces through DRAM.
  Phase B (per 128-row query tile, software-pipelined): scores = Q^T.T @ K^T
           causal chunks -> additive tril mask on the diagonal block ->
           rowmax/exp/rowsum on ACT -> PE-transpose P -> P^T @ V accumulation
           -> scale by 1/rowsum, add bv, store.
"""

import os
import sys

sys.path.insert(0, "/opt/trn_rl_repo")

import numpy as np

import concourse.bacc as bacc
import concourse.tile as tile
from concourse import mybir
from concourse.bass import ds, ts
import concourse.bass as bass
from concourse.bass_utils import run_bass_kernel_spmd

F32 = mybir.dt.float32
F32R = mybir.dt.float32r

B, S, D = 8, 2048, 1024
P = 128                     # partition width
DT = D // P                 # 8 d-tiles (contraction)
ET = D // P                 # 8 e-tiles (output feature tiles)
ST = S // P                 # 16 s-tiles
GROUP_S = 256               # s-rows per phase-A group
NG = S // GROUP_S           # 8 groups
NEG = -1.0e30

USE_F32R = os.environ.get("ATTN_NO_F32R", "") == ""
MM_DT = F32R if USE_F32R else F32


def _build(nc):
    xq = nc.declare_dram_parameter("xq", [S, D], F32, isOutput=False)
    xk = nc.declare_dram_parameter("xk", [S, D], F32, isOutput=False)
    xv = nc.declare_dram_parameter("xv", [S, D], F32, isOutput=False)
    wq = nc.declare_dram_parameter("wq", [D, D], F32, isOutput=False)
    wk = nc.declare_dram_parameter("wk", [D, D], F32, isOutput=False)
    wv = nc.declare_dram_parameter("wv", [D, D], F32, isOutput=False)
    bq = nc.declare_dram_parameter("bq", [D], F32, isOutput=False)
    bk = nc.declare_dram_parameter("bk", [D], F32, isOutput=False)
    bv = nc.declare_dram_parameter("bv", [D], F32, isOutput=False)
    # [128, 512] additive mask; cols 384..511 hold the tril block, rest 0
    maskc = nc.declare_dram_parameter("maskc", [P, 512], F32, isOutput=False)
    ident = nc.declare_dram_parameter("ident", [P, P], F32, isOutput=False)
    out_ext = nc.declare_dram_parameter("out", [S, D], F32, isOutput=True)

    qt_dram = nc.dram_tensor("qt_bounce", [P, ET, S], MM_DT)

    with tile.TileContext(nc) as tc:
        with (
            tc.tile_pool(name="res", bufs=1) as res,          # long-lived
            tc.tile_pool(name="ps_tr", bufs=2, space="PSUM") as ps_tr,
            tc.tile_pool(name="ps_mm", bufs=4, space="PSUM") as ps_mm,
            tc.tile_pool(name="ps_pv", bufs=2, space="PSUM") as ps_pv,
        ):
            kt_sb = res.tile([P, ET, S], MM_DT, tag="kt")     # K^T [e, sk]
            v_sb = res.tile([P, ST, D], MM_DT, tag="v")       # V [sk, dv]

            ident_sb = res.tile([P, P], F32, tag="ident")
            nc.sync.dma_start(out=ident_sb, in_=ident[:, :])
            maskc_sb = res.tile([P, 512], F32, tag="maskc")
            nc.sync.dma_start(out=maskc_sb, in_=maskc[:, :])

            bias_sb = res.tile([P, 3, ET], F32, tag="bias")   # bk | bq/32 | unused
            for e in range(ET):
                nc.sync.dma_start(out=bias_sb[:, 0, ds(e, 1)], in_=bk[ts(e, P)])
                nc.sync.dma_start(out=bias_sb[:, 2, ds(e, 1)], in_=bq[ts(e, P)])
            nc.scalar.mul(
                out=bias_sb[:, 1, :], in_=bias_sb[:, 2, :], mul=1.0 / np.sqrt(D)
            )

            bv_sb = res.tile([P, D], F32, tag="bv")
            bv_ap = bv[:]
            bv_bcast = bass.AP(
                tensor=bv_ap.tensor, offset=bv_ap.offset, ap=[[0, P], [1, D]]
            )
            nc.gpsimd.dma_start(out=bv_sb, in_=bv_bcast)

            # ---------------- Phase A: projections ----------------
            with tc.tile_pool(name="pha", bufs=1) as pha:
                wr = pha.tile([P, DT, D], MM_DT, tag="wr")
                for proj, x_ext, w_ext in (
                    ("k", xk, wk),
                    ("v", xv, wv),
                    ("q", xq, wq),
                ):
                    for d in range(DT):
                        wraw = pha.tile([P, D], F32, tag="wraw")
                        nc.sync.dma_start(out=wraw, in_=w_ext[ts(d, P), :])
                        nc.scalar.copy(out=wr[:, d, :], in_=wraw)

                    for g in range(NG):
                        # transpose X rows [g*256, g*256+256) -> XT [d, 256]
                        xt_t = pha.tile([P, DT, GROUP_S], MM_DT, tag="xt")
                        for ss in range(GROUP_S // P):
                            xnat = pha.tile([P, D], F32, tag="xnat")
                            nc.sync.dma_start(
                                out=xnat,
                                in_=x_ext[ds(g * GROUP_S + ss * P, P), :],
                            )
                            for db in range(DT // 4):
                                trp = ps_tr.tile([P, 512], F32, tag="tr")
                                for k4 in range(4):
                                    nc.tensor.transpose(
                                        out=trp[:, ts(k4, P)],
                                        in_=xnat[:, ts(db * 4 + k4, P)],
                                        identity=ident_sb,
                                    )
                                nc.vector.tensor_copy(
                                    out=xt_t[:, ds(db * 4, 4), ts(ss, P)],
                                    in_=trp[:, :].rearrange(
                                        "p (a b) -> p a b", a=4
                                    ),
                                )

                        if proj == "v":
                            for ss in range(GROUP_S // P):
                                t_idx = g * (GROUP_S // P) + ss
                                for dv in range(2):
                                    vp = ps_mm.tile([P, 512], F32, tag="mm")
                                    for d in range(DT):
                                        nc.tensor.matmul(
                                            vp,
                                            xt_t[:, d, ts(ss, P)],
                                            wr[:, d, ts(dv, 512)],
                                            start=(d == 0),
                                            stop=(d == DT - 1),
                                        )
                                    nc.scalar.copy(
                                        out=v_sb[:, t_idx, ts(dv, 512)], in_=vp
                                    )
                        else:
                            for e in range(ET):
                                pp = ps_mm.tile([P, GROUP_S], F32, tag="mm")
                                for d in range(DT):
                                    nc.tensor.matmul(
                                        pp,
                                        wr[:, d, ts(e, P)],
                                        xt_t[:, d, :],
                                        start=(d == 0),
                                        stop=(d == DT - 1),
                                    )
                                if proj == "k":
                                    nc.scalar.activation(
                                        out=kt_sb[:, e, ds(g * GROUP_S, GROUP_S)],
                                        in_=pp,
                                        func=mybir.ActivationFunctionType.Identity,
                                        bias=bias_sb[:, 0, ds(e, 1)],
                                        scale=1.0,
                                    )
                                else:  # q: scale by 1/sqrt(D), bias bq/sqrt(D)
                                    qt_stage = pha.tile(
                                        [P, GROUP_S], MM_DT, tag="qstage"
                                    )
                                    nc.scalar.activation(
                                        out=qt_stage,
                                        in_=pp,
                                        func=mybir.ActivationFunctionType.Identity,
                                        bias=bias_sb[:, 1, ds(e, 1)],
                                        scale=float(1.0 / np.sqrt(D)),
                                    )
                                    nc.sync.dma_start(
                                        out=qt_dram[:, e, ds(g * GROUP_S, GROUP_S)],
                                        in_=qt_stage,
                                    )

            # ---------------- Phase B: attention ----------------
            with tc.tile_pool(name="phb", bufs=1) as phb:

                def softmax_part(i):
                    """scores + softmax for q-tile i; returns (p_sb, rl, n_k)."""
                    L = (i + 1) * P
                    n_chunks = (L + 511) // 512
                    qt_t = phb.tile([P, ET, P], MM_DT, tag="qt")
                    nc.sync.dma_start(out=qt_t, in_=qt_dram[:, :, ts(i, P)])

                    sc_sb = phb.tile([P, S], F32, tag="scores")
                    for c in range(n_chunks):
                        cs = c * 512
                        w = min(512, L - cs)
                        sp = ps_mm.tile([P, 512], F32, tag="mm")
                        for e in range(ET):
                            nc.tensor.matmul(
                                sp[:, :w],
                                qt_t[:, e, :],
                                kt_sb[:, e, ds(cs, w)],
                                start=(e == 0),
                                stop=(e == ET - 1),
                            )
                        if c == n_chunks - 1:
                            nc.vector.tensor_add(
                                out=sc_sb[:, ds(cs, w)],
                                in0=sp[:, :w],
                                in1=maskc_sb[:, ds(512 - w, w)],
                            )
                        else:
                            nc.vector.tensor_copy(
                                out=sc_sb[:, ds(cs, w)], in_=sp[:, :w]
                            )

                    stats = phb.tile([P, 4], F32, tag="stats")
                    nc.vector.reduce_max(
                        out=stats[:, 0:1],
                        in_=sc_sb[:, :L],
                        axis=mybir.AxisListType.X,
                        negate=True,
                    )
                    p_sb = phb.tile([P, S], MM_DT, tag="p")
                    nc.scalar.activation(
                        out=p_sb[:, :L],
                        in_=sc_sb[:, :L],
                        func=mybir.ActivationFunctionType.Exp,
                        bias=stats[:, 0:1],
                        scale=1.0,
                        accum_out=stats[:, 1:2],
                    )
                    nc.vector.reciprocal(out=stats[:, 2:3], in_=stats[:, 1:2])
                    return p_sb, stats

                def pv_part(i, p_sb, stats):
                    """P^T, P^T @ V, normalize, +bv, store for q-tile i."""
                    n_k = i + 1
                    pt_t = phb.tile([P, ST, P], MM_DT, tag="pt")
                    for tb in range((n_k + 3) // 4):
                        nb = min(4, n_k - tb * 4)
                        trp = ps_tr.tile([P, 512], F32, tag="tr")
                        for k4 in range(nb):
                            nc.tensor.transpose(
                                out=trp[:, ts(k4, P)],
                                in_=p_sb[:, ts(tb * 4 + k4, P)].bitcast(F32),
                                identity=ident_sb,
                            )
                        nc.scalar.copy(
                            out=pt_t[:, ds(tb * 4, nb), :],
                            in_=trp[:, ds(0, nb * P)].rearrange(
                                "p (a b) -> p a b", a=nb
                            ),
                        )

                    out_sb = phb.tile([P, D], F32, tag="osb")
                    for dv in range(2):
                        pvp = ps_pv.tile([P, 512], F32, tag="pv")
                        for t in range(n_k):
                            nc.tensor.matmul(
                                pvp,
                                pt_t[:, t, :],
                                v_sb[:, t, ts(dv, 512)],
                                start=(t == 0),
                                stop=(t == n_k - 1),
                            )
                        nc.vector.tensor_scalar_mul(
                            out=out_sb[:, ts(dv, 512)],
                            in0=pvp,
                            scalar1=stats[:, 2:3],
                        )
                        nc.vector.tensor_add(
                            out=out_sb[:, ts(dv, 512)],
                            in0=out_sb[:, ts(dv, 512)],
                            in1=bv_sb[:, ts(dv, 512)],
                        )
                    nc.sync.dma_start(out=out_ext[ts(i, P), :], in_=out_sb)

                prev = None
                for i in range(ST):
                    cur = (i, *softmax_part(i))
                    if prev is not None:
                        pv_part(*prev)
                    prev = cur
                pv_part(*prev)

    nc.compile()
    return nc


_NC_CACHE = None


def _get_nc():
    global _NC_CACHE
    if _NC_CACHE is None:
        nc = bacc.Bacc("TRN2", target_bir_lowering=False)
        _NC_CACHE = _build(nc)
    return _NC_CACHE


def _host_inputs(query, key, value, mask, Wq, bq, Wk, bk, Wv, bv):
    tril = np.tril(np.ones((S, S), dtype=bool))
    if not np.array_equal(np.asarray(mask, dtype=bool), tril):
        raise ValueError("kernel is specialized to the causal (tril) mask")

    row = np.arange(P)[:, None]
    col = np.arange(P)[None, :]
    tril_add = np.where(row >= col, 0.0, NEG).astype(np.float32)
    maskc = np.concatenate(
        [np.zeros((P, 512 - P), np.float32), tril_add], axis=1
    )
    ident = np.eye(P, dtype=np.float32)

    shared = {
        "wq": np.ascontiguousarray(Wq, np.float32),
        "wk": np.ascontiguousarray(Wk, np.float32),
        "wv": np.ascontiguousarray(Wv, np.float32),
        "bq": np.ascontiguousarray(bq, np.float32),
        "bk": np.ascontiguousarray(bk, np.float32),
        "bv": np.ascontiguousarray(bv, np.float32),
        "maskc": maskc,
        "ident": ident,
    }
    in_maps = []
    for b in range(B):
        m = dict(shared)
        m["xq"] = np.ascontiguousarray(query[b], np.float32)
        m["xk"] = np.ascontiguousarray(key[b], np.float32)
        m["xv"] = np.ascontiguousarray(value[b], np.float32)
        in_maps.append(m)
    return in_maps


def run(inputs, trace=False, **spmd_kwargs):
    nc = _get_nc()
    in_maps = _host_inputs(**inputs)
    res = run_bass_kernel_spmd(
        nc, in_maps, list(range(B)), trace=trace, **spmd_kwargs
    )
    out = np.stack([res.results[c]["out"] for c in range(B)], axis=0)
    return out.astype(np.float32), res


def kernel(**inputs) -> np.ndarray:
    out, _ = run(inputs, trace=False)
    return out


# revision 4
# speedup vs baseline: 86.4804x; 86.4804x over previous
"""Single-head causal attention (B=8, S=2048, D=1024) on 8 Trainium2 cores.

Strategy: pure data-parallel over batch — core b computes attention for
batch element b end-to-end (no collectives). All matmuls run in FP32r
(fp32 with 11-bit mantissa, full PE rate at moving-dim >= 256).

Per-core pipeline:
  Phase A: PE-transpose activations, project K^T (+bk), V, Q^T (+bq, /sqrt(D))
           with f32r matmuls. K^T [e, sk] and V [sk, dv] stay resident in
           SBUF; Q^T bounces through DRAM.
  Phase B (per 128-row query tile, software-pipelined): scores = Q^T.T @ K^T
           causal chunks -> additive tril mask on the diagonal block ->
           rowmax/exp/rowsum on ACT -> PE-transpose P -> P^T @ V accumulation
           -> scale by 1/rowsum, add bv, store.
"""

import os
import sys

sys.path.insert(0, "/opt/trn_rl_repo")

import numpy as np

import concourse.bacc as bacc
import concourse.tile as tile
from concourse import mybir
from concourse.bass import ds, ts
import concourse.bass as bass
from concourse.bass_utils import run_bass_kernel_spmd

F32 = mybir.dt.float32
F32R = mybir.dt.float32r

B, S, D = 8, 2048, 1024
P = 128                     # partition width
DT = D // P                 # 8 d-tiles (contraction)
ET = D // P                 # 8 e-tiles (output feature tiles)
ST = S // P                 # 16 s-tiles
GROUP_S = 256               # s-rows per phase-A group
NG = S // GROUP_S           # 8 groups
NEG = -1.0e30

USE_F32R = os.environ.get("ATTN_NO_F32R", "") == ""
MM_DT = F32R if USE_F32R else F32


def _phase_a(nc, tc, ext, consts, kt_sb, v_sb, qt_dram, ps_tr, ps_mm):
    """Projections: fill kt_sb, v_sb (SBUF) and qt_dram (DRAM bounce)."""
    ident_sb, maskc_sb, bias_sb, bv_sb = consts
    with tc.tile_pool(name="pha", bufs=1) as pha:
        wr = pha.tile([P, DT, D], MM_DT, tag="wr")
        for proj in ("k", "v", "q"):
            x_ext = ext["x" + proj]
            w_ext = ext["w" + proj]
            for d in range(DT):
                wraw = pha.tile([P, D], F32, tag="wraw")
                nc.sync.dma_start(out=wraw, in_=w_ext[ts(d, P), :])
                nc.scalar.copy(out=wr[:, d, :], in_=wraw)

            for g in range(NG):
                # transpose X rows [g*256, g*256+256) -> XT [d, 256]
                xt_t = pha.tile([P, DT, GROUP_S], MM_DT, tag="xt")
                for ss in range(GROUP_S // P):
                    xnat = pha.tile([P, D], F32, tag="xnat")
                    nc.sync.dma_start(
                        out=xnat, in_=x_ext[ds(g * GROUP_S + ss * P, P), :]
                    )
                    for db in range(DT // 4):
                        trp = ps_tr.tile([P, 512], F32, tag="tr")
                        for k4 in range(4):
                            nc.tensor.transpose(
                                out=trp[:, ts(k4, P)],
                                in_=xnat[:, ts(db * 4 + k4, P)],
                                identity=ident_sb,
                            )
                        nc.vector.tensor_copy(
                            out=xt_t[:, ds(db * 4, 4), ts(ss, P)],
                            in_=trp[:, :].rearrange("p (a b) -> p a b", a=4),
                        )

                if proj == "v":
                    for ss in range(GROUP_S // P):
                        t_idx = g * (GROUP_S // P) + ss
                        for dv in range(2):
                            vp = ps_mm.tile([P, 512], F32, tag="mm")
                            for d in range(DT):
                                nc.tensor.matmul(
                                    vp,
                                    xt_t[:, d, ts(ss, P)],
                                    wr[:, d, ts(dv, 512)],
                                    start=(d == 0),
                                    stop=(d == DT - 1),
                                )
                            nc.scalar.copy(
                                out=v_sb[:, t_idx, ts(dv, 512)], in_=vp
                            )
                else:
                    for e in range(ET):
                        pp = ps_mm.tile([P, GROUP_S], F32, tag="mm")
                        for d in range(DT):
                            nc.tensor.matmul(
                                pp,
                                wr[:, d, ts(e, P)],
                                xt_t[:, d, :],
                                start=(d == 0),
                                stop=(d == DT - 1),
                            )
                        if proj == "k":
                            nc.scalar.activation(
                                out=kt_sb[:, e, ds(g * GROUP_S, GROUP_S)],
                                in_=pp,
                                func=mybir.ActivationFunctionType.Identity,
                                bias=bias_sb[:, 0, ds(e, 1)],
                                scale=1.0,
                            )
                        else:  # q: scale by 1/sqrt(D), bias bq/sqrt(D)
                            qt_stage = pha.tile([P, GROUP_S], MM_DT, tag="qstage")
                            nc.scalar.activation(
                                out=qt_stage,
                                in_=pp,
                                func=mybir.ActivationFunctionType.Identity,
                                bias=bias_sb[:, 1, ds(e, 1)],
                                scale=float(1.0 / np.sqrt(D)),
                            )
                            nc.sync.dma_start(
                                out=qt_dram[:, e, ds(g * GROUP_S, GROUP_S)],
                                in_=qt_stage,
                            )


def _phase_b(nc, tc, out_ext, consts, kt_sb, v_sb, qt_dram, ps_tr, ps_mm, ps_pv):
    ident_sb, maskc_sb, bias_sb, bv_sb = consts
    with tc.tile_pool(name="phb", bufs=1) as phb:

        def softmax_part(i):
            """scores + softmax for q-tile i; returns (p_sb, stats)."""
            L = (i + 1) * P
            n_chunks = (L + 511) // 512
            qt_t = phb.tile([P, ET, P], MM_DT, tag="qt")
            nc.sync.dma_start(out=qt_t, in_=qt_dram[:, :, ts(i, P)])

            sc_sb = phb.tile([P, S], F32, tag="scores")
            for c in range(n_chunks):
                cs = c * 512
                w = min(512, L - cs)
                sp = ps_mm.tile([P, 512], F32, tag="mm")
                for e in range(ET):
                    nc.tensor.matmul(
                        sp[:, :w],
                        qt_t[:, e, :],
                        kt_sb[:, e, ds(cs, w)],
                        start=(e == 0),
                        stop=(e == ET - 1),
                    )
                if c == n_chunks - 1:
                    nc.vector.tensor_add(
                        out=sc_sb[:, ds(cs, w)],
                        in0=sp[:, :w],
                        in1=maskc_sb[:, ds(512 - w, w)],
                    )
                else:
                    nc.vector.tensor_copy(out=sc_sb[:, ds(cs, w)], in_=sp[:, :w])

            stats = phb.tile([P, 4], F32, tag="stats")
            nc.vector.reduce_max(
                out=stats[:, 0:1],
                in_=sc_sb[:, :L],
                axis=mybir.AxisListType.X,
                negate=True,
            )
            p_sb = phb.tile([P, S], MM_DT, tag="p")
            nc.scalar.activation(
                out=p_sb[:, :L],
                in_=sc_sb[:, :L],
                func=mybir.ActivationFunctionType.Exp,
                bias=stats[:, 0:1],
                scale=1.0,
                accum_out=stats[:, 1:2],
            )
            nc.vector.reciprocal(out=stats[:, 2:3], in_=stats[:, 1:2])
            return p_sb, stats

        def pv_part(i, p_sb, stats):
            """P^T, P^T @ V, normalize, +bv, store for q-tile i."""
            n_k = i + 1
            pt_t = phb.tile([P, ST, P], MM_DT, tag="pt")
            for tb in range((n_k + 3) // 4):
                nb = min(4, n_k - tb * 4)
                trp = ps_tr.tile([P, 512], F32, tag="tr")
                for k4 in range(nb):
                    nc.tensor.transpose(
                        out=trp[:, ts(k4, P)],
                        in_=p_sb[:, ts(tb * 4 + k4, P)].bitcast(F32),
                        identity=ident_sb,
                    )
                nc.scalar.copy(
                    out=pt_t[:, ds(tb * 4, nb), :],
                    in_=trp[:, ds(0, nb * P)].rearrange("p (a b) -> p a b", a=nb),
                )

            out_sb = phb.tile([P, D], F32, tag="osb")
            for dv in range(2):
                pvp = ps_pv.tile([P, 512], F32, tag="pv")
                for t in range(n_k):
                    nc.tensor.matmul(
                        pvp,
                        pt_t[:, t, :],
                        v_sb[:, t, ts(dv, 512)],
                        start=(t == 0),
                        stop=(t == n_k - 1),
                    )
                nc.vector.tensor_scalar_mul(
                    out=out_sb[:, ts(dv, 512)], in0=pvp, scalar1=stats[:, 2:3]
                )
                nc.vector.tensor_add(
                    out=out_sb[:, ts(dv, 512)],
                    in0=out_sb[:, ts(dv, 512)],
                    in1=bv_sb[:, ts(dv, 512)],
                )
            nc.sync.dma_start(out=out_ext[ts(i, P), :], in_=out_sb)

        prev = None
        for i in range(ST):
            cur = (i, *softmax_part(i))
            if prev is not None:
                pv_part(*prev)
            prev = cur
        pv_part(*prev)


def _build(nc, repeat=1):
    ext = {}
    ext["xq"] = nc.declare_dram_parameter("xq", [S, D], F32, isOutput=False)
    ext["xk"] = nc.declare_dram_parameter("xk", [S, D], F32, isOutput=False)
    ext["xv"] = nc.declare_dram_parameter("xv", [S, D], F32, isOutput=False)
    ext["wq"] = nc.declare_dram_parameter("wq", [D, D], F32, isOutput=False)
    ext["wk"] = nc.declare_dram_parameter("wk", [D, D], F32, isOutput=False)
    ext["wv"] = nc.declare_dram_parameter("wv", [D, D], F32, isOutput=False)
    bq = nc.declare_dram_parameter("bq", [D], F32, isOutput=False)
    bk = nc.declare_dram_parameter("bk", [D], F32, isOutput=False)
    bv = nc.declare_dram_parameter("bv", [D], F32, isOutput=False)
    # [128, 512] additive mask; cols 384..511 hold the tril block, rest 0
    maskc = nc.declare_dram_parameter("maskc", [P, 512], F32, isOutput=False)
    ident = nc.declare_dram_parameter("ident", [P, P], F32, isOutput=False)
    out_ext = nc.declare_dram_parameter("out", [S, D], F32, isOutput=True)

    qt_dram = nc.dram_tensor("qt_bounce", [P, ET, S], MM_DT)

    with tile.TileContext(nc) as tc:
        with (
            tc.tile_pool(name="res", bufs=1) as res,          # long-lived
            tc.tile_pool(name="ps_tr", bufs=2, space="PSUM") as ps_tr,
            tc.tile_pool(name="ps_mm", bufs=4, space="PSUM") as ps_mm,
            tc.tile_pool(name="ps_pv", bufs=2, space="PSUM") as ps_pv,
        ):
            kt_sb = res.tile([P, ET, S], MM_DT, tag="kt")     # K^T [e, sk]
            v_sb = res.tile([P, ST, D], MM_DT, tag="v")       # V [sk, dv]

            ident_sb = res.tile([P, P], F32, tag="ident")
            nc.sync.dma_start(out=ident_sb, in_=ident[:, :])
            maskc_sb = res.tile([P, 512], F32, tag="maskc")
            nc.sync.dma_start(out=maskc_sb, in_=maskc[:, :])

            bias_sb = res.tile([P, 3, ET], F32, tag="bias")   # bk | bq/32 | raw bq
            for e in range(ET):
                nc.sync.dma_start(out=bias_sb[:, 0, ds(e, 1)], in_=bk[ts(e, P)])
                nc.sync.dma_start(out=bias_sb[:, 2, ds(e, 1)], in_=bq[ts(e, P)])
            nc.scalar.mul(
                out=bias_sb[:, 1, :], in_=bias_sb[:, 2, :], mul=1.0 / np.sqrt(D)
            )

            bv_sb = res.tile([P, D], F32, tag="bv")
            bv_ap = bv[:]
            bv_bcast = bass.AP(
                tensor=bv_ap.tensor, offset=bv_ap.offset, ap=[[0, P], [1, D]]
            )
            nc.gpsimd.dma_start(out=bv_sb, in_=bv_bcast)

            consts = (ident_sb, maskc_sb, bias_sb, bv_sb)
            for _rep in range(repeat):
                _phase_a(nc, tc, ext, consts, kt_sb, v_sb, qt_dram, ps_tr, ps_mm)
                _phase_b(
                    nc, tc, out_ext, consts, kt_sb, v_sb, qt_dram,
                    ps_tr, ps_mm, ps_pv,
                )

    nc.compile()
    return nc


_NC_CACHE = {}


def _get_nc(repeat=1):
    if repeat not in _NC_CACHE:
        nc = bacc.Bacc("TRN2", target_bir_lowering=False)
        _NC_CACHE[repeat] = _build(nc, repeat=repeat)
    return _NC_CACHE[repeat]


def _host_inputs(query, key, value, mask, Wq, bq, Wk, bk, Wv, bv):
    tril = np.tril(np.ones((S, S), dtype=bool))
    if not np.array_equal(np.asarray(mask, dtype=bool), tril):
        raise ValueError("kernel is specialized to the causal (tril) mask")

    row = np.arange(P)[:, None]
    col = np.arange(P)[None, :]
    tril_add = np.where(row >= col, 0.0, NEG).astype(np.float32)
    maskc = np.concatenate(
        [np.zeros((P, 512 - P), np.float32), tril_add], axis=1
    )
    ident = np.eye(P, dtype=np.float32)

    shared = {
        "wq": np.ascontiguousarray(Wq, np.float32),
        "wk": np.ascontiguousarray(Wk, np.float32),
        "wv": np.ascontiguousarray(Wv, np.float32),
        "bq": np.ascontiguousarray(bq, np.float32),
        "bk": np.ascontiguousarray(bk, np.float32),
        "bv": np.ascontiguousarray(bv, np.float32),
        "maskc": maskc,
        "ident": ident,
    }
    in_maps = []
    for b in range(B):
        m = dict(shared)
        m["xq"] = np.ascontiguousarray(query[b], np.float32)
        m["xk"] = np.ascontiguousarray(key[b], np.float32)
        m["xv"] = np.ascontiguousarray(value[b], np.float32)
        in_maps.append(m)
    return in_maps


def run(inputs, trace=False, repeat=1, **spmd_kwargs):
    nc = _get_nc(repeat)
    in_maps = _host_inputs(**inputs)
    res = run_bass_kernel_spmd(
        nc, in_maps, list(range(B)), trace=trace, **spmd_kwargs
    )
    out = np.stack([res.results[c]["out"] for c in range(B)], axis=0)
    return out.astype(np.float32), res


def kernel(**inputs) -> np.ndarray:
    out, _ = run(inputs, trace=False)
    return out
